# revision 54
# baseline (speedup 1.0000x reference)
"""Multi-head attention kernel for 8 Trainium2 NeuronCores.

Strategy: tensor-parallel over heads. Core c owns heads (2c, 2c+1), i.e.
columns [128c, 128c+128) of the projection space.
  - column-parallel Wq/Wk/Wv: each core projects the full token stream onto
    its 128 columns; q/k are produced transposed ([cols, tok]) so the
    attention matmuls contract over the partition dim natively.
  - scores^T = k^T_blk.T @ q^T with softmax along the key dim (= partition).
    The two heads' score matmuls run CONCURRENTLY on the PE array (row
    tiling: head0 owns partitions 0-63, head1 owns 64-127, auto
    tile_position from base_partition).
  - normalization deferred: E = exp(scale*scores + mask_bias),
    U^T = v.T @ E with an appended ones row giving sum(E) for free;
    ctx^T = U^T / sum via reciprocal_approx_fast on the [1,512] sum row +
    GPSIMD partition_broadcast + one DVE mul.
  - row-parallel Wo: each core emits a partial [4096, 1024] fp16 output;
    the host sums the 8 partials and adds bo.

Schedule: the ScalarE exp stream is the bottleneck (~143us busy) and paces
the kernel. Attention for batch 0 starts as soon as kT (both halves) and
the first 512-token qT chunk are projected (~20us in). The U-matmul stream
lags the score/exp stream through a pending queue so exp never waits on the
v projection; the lag is scheduled so v blocks arrive just in time. All
remaining projections, v transposes and output projections are emitted as
background PE work gated to data arrival and driven in the exp-paced slack.

Matmul operands are fp16 (PE 1 cycle/row, FWL weight loads); accumulation
is fp32 in PSUM. Inputs are pre-transposed and cast to fp16 on the host.
"""

import collections

import numpy as np

import concourse.bass as bass
import concourse.tile as tile
from concourse import bacc, library_config, mybir
from concourse.bass_utils import run_bass_kernel_spmd

B, S, D, H = 2, 2048, 1024, 16
DH = D // H          # 64
NCORES = 8
HPC = H // NCORES    # heads per core = 2
CW = HPC * DH        # column width per core = 128
T = B * S            # 4096 tokens
SCALE = 1.0 / np.sqrt(DH)

F32 = mybir.dt.float32
F16 = mybir.dt.float16

# v_s block layout: per 128-token block: [v_h0 (64) | ones | v_h1 (64) | ones]
VBLK = 2 * (DH + 1)  # 130

NKT = D // 128       # 8 contraction tiles for projections
NQC = S // 512       # 4 q-chunks per batch
NKB = S // 128       # 16 key blocks per batch
NTB = S // 128       # 16 token blocks per batch


def build_nc(zero_bias=True, dbg=False):
    nc = bacc.Bacc("TRN2", target_bir_lowering=False, debug=False,
                   num_devices=NCORES)

    qT_d = nc.declare_dram_parameter("qT", [D, T], F16, isOutput=False)
    kT_d = nc.declare_dram_parameter("kT", [D, T], F16, isOutput=False)
    vT_d = nc.declare_dram_parameter("vT", [D, T], F16, isOutput=False)
    # projection weights arrive host-permuted: w[p, kt*CW+m] = W[kt*128+p, m]
    wq_d = nc.declare_dram_parameter("wq", [128, NKT * CW], F16,
                                     isOutput=False)
    wk_d = nc.declare_dram_parameter("wk", [128, NKT * CW], F16,
                                     isOutput=False)
    wv_d = nc.declare_dram_parameter("wv", [128, NKT * CW], F16,
                                     isOutput=False)
    wo_d = nc.declare_dram_parameter("wo", [CW, D], F16, isOutput=False)
    bqkv_d = nc.declare_dram_parameter("bqkv", [CW, 3], F32, isOutput=False)
    maskb_d = nc.declare_dram_parameter("maskb", [128, B * NKB], F32,
                                        isOutput=False)
    ident_d = nc.declare_dram_parameter("ident", [128, 128], F16,
                                        isOutput=False)
    out_d = nc.declare_dram_parameter("out", [T, D], F16, isOutput=True)
    if dbg:
        dbg_d = nc.declare_dram_parameter("dbg", [128, 4 * S], F16,
                                          isOutput=True)

    with tile.TileContext(nc) as tc:
        with (
            tc.tile_pool(name="weights", bufs=1) as wpool,
            tc.tile_pool(name="resident", bufs=1) as rpool,
            tc.tile_pool(name="inK", bufs=16) as inK,
            tc.tile_pool(name="inQ", bufs=16) as inQ,
            tc.tile_pool(name="vt_tmp", bufs=2) as vtpool,
            tc.tile_pool(name="E", bufs=24) as epool,
            tc.tile_pool(name="r", bufs=3) as recpool,
            tc.tile_pool(name="Rsb", bufs=3) as rsbpool,
            tc.tile_pool(name="outsb", bufs=3) as outpool,
            # PSUM (8 banks): psA 2x[128,1024] = 4, psP 2x[128,512] = 2,
            # psU 2x[65,512] = 2
            tc.tile_pool(name="psA", bufs=2, space="PSUM") as psapool,
            tc.tile_pool(name="psP", bufs=2, space="PSUM") as psppool,
            tc.tile_pool(name="psU", bufs=2, space="PSUM") as psupool,
        ):
            nc.gpsimd.load_library(library_config.attn)

            # ---- load weights / constants (SBUF-resident) ----
            # Critical path first: wk/wq (+ tiny maskb) gate the first
            # scores; wv/wo/ident aren't needed until much later.
            # w*_s[p, kt*CW + m] = w[kt*128 + p, m]
            wq_s = wpool.tile([128, NKT * CW], F16, tag="wq")
            wk_s = wpool.tile([128, NKT * CW], F16, tag="wk")
            wv_s = wpool.tile([128, NKT * CW], F16, tag="wv")

            def load_w(w_s, w_d):
                nc.sync.dma_start(w_s[:], w_d[:, :])

            load_w(wk_s, wk_d)
            load_w(wq_s, wq_d)
            maskb_s = wpool.tile([128, B * NKB], F32, tag="maskb")
            nc.sync.dma_start(maskb_s[:], maskb_d[:, :])
            bqkv_s = wpool.tile([CW, 3], F32, tag="bqkv")
            if not zero_bias:
                nc.sync.dma_start(bqkv_s[:], bqkv_d[:, :])
            # preload the exp table set (~2.7us) while DMA streams in
            warm_in = wpool.tile([1, 16], F32, tag="warm_in")
            warm_out = wpool.tile([1, 16], F32, tag="warm_out")
            nc.vector.memset(warm_in[:], 0.0)
            nc.scalar.activation(warm_out[:], warm_in[:],
                                 mybir.ActivationFunctionType.Exp)
            wo_s = wpool.tile([128, D], F16, tag="wo")
            ident_s = wpool.tile([128, 128], F16, tag="ident")

            def wv_ident_gen():
                load_w(wv_s, wv_d)
                nc.sync.dma_start(ident_s[:], ident_d[:, :])
                return
                yield

            def wo_gen():
                nc.sync.dma_start(wo_s[:], wo_d[:, :])
                return
                yield

            # ---- per-batch resident activation tiles ----
            qT_s = [rpool.tile([128, S], F16, tag=f"qT{b}", name=f"qT_s{b}")
                    for b in range(B)]
            kT_s = [rpool.tile([128, S], F16, tag=f"kT{b}", name=f"kT_s{b}")
                    for b in range(B)]
            v_s = [rpool.tile([128, NTB * VBLK], F16, tag=f"v{b}",
                              name=f"v_s{b}") for b in range(B)]
            ctxT_s = [rpool.tile([128, S], F16, tag=f"ctxT{b}",
                                 name=f"ctxT_s{b}") for b in range(B)]
            vt_tmp = [vtpool.tile([128, S], F16, tag="vt_tmp",
                                  name=f"vt_tmp{b}") for b in range(B)]

            for b in range(B):
                # ones columns interleaved into the v layout
                nc.vector.memset(
                    v_s[b][:].rearrange("p (k j) -> p k j", j=DH + 1)
                    [:, :, DH], 1.0)

            def drain_acc(dst, acc, bias_col):
                if zero_bias:
                    nc.vector.tensor_copy(dst, acc[:])
                else:
                    nc.vector.tensor_scalar_add(dst, acc[:],
                                                bqkv_s[:, bias_col:bias_col + 1])

            # ---- projection emitters (yield once per PE instruction) ----
            def qproj(b, qc):
                # one 512-token chunk of the q projection
                srcs = []
                for kt in range(NKT):
                    t = inQ.tile([128, 512], F16, tag="inQ",
                                 name=f"qsrc{b}_{qc}_{kt}")
                    nc.sync.dma_start(
                        t[:], qT_d[kt * 128:(kt + 1) * 128,
                                   b * S + qc * 512:b * S + (qc + 1) * 512])
                    srcs.append(t)
                acc = psppool.tile([128, 512], F32, tag="psP",
                                   name=f"qacc{b}_{qc}")
                for kt in range(NKT):
                    nc.tensor.matmul(
                        acc[:], wq_s[:, kt * CW:(kt + 1) * CW], srcs[kt][:],
                        start=(kt == 0), stop=(kt == NKT - 1))
                    yield
                drain_acc(qT_s[b][:, qc * 512:(qc + 1) * 512], acc, 0)

            def kvproj(b, pp, which):
                # one 1024-token half of the k or v projection (j-serial)
                w_s, dst_s, bias_col = (
                    (wk_s, kT_s[b], 1) if which == "k"
                    else (wv_s, vt_tmp[b], 2))
                src_d = kT_d if which == "k" else vT_d
                srcs = []
                for kt in range(NKT):
                    t = inK.tile([128, 1024], F16, tag="inK",
                                 name=f"{which}src{b}_{pp}_{kt}")
                    nc.sync.dma_start(
                        t[:], src_d[kt * 128:(kt + 1) * 128,
                                    b * S + pp * 1024:b * S + (pp + 1) * 1024])
                    srcs.append(t)
                for j in range(2):
                    acc = psppool.tile([128, 512], F32, tag="psP",
                                       name=f"{which}acc{b}_{pp}_{j}")
                    for kt in range(NKT):
                        nc.tensor.matmul(
                            acc[:], w_s[:, kt * CW:(kt + 1) * CW],
                            srcs[kt][:, j * 512:(j + 1) * 512],
                            start=(kt == 0), stop=(kt == NKT - 1))
                        yield
                    drain_acc(
                        dst_s[:, pp * 1024 + j * 512:pp * 1024 + (j + 1) * 512],
                        acc, bias_col)
                    if which == "v":
                        # PE-transpose this 512-token chunk into v_s layout
                        for t in range(pp * 8 + j * 4, pp * 8 + (j + 1) * 4):
                            pst = psapool.tile([128, 128], F16, tag="psA",
                                               name=f"pst{b}_{t}")
                            nc.tensor.transpose(
                                pst[:], vt_tmp[b][:, t * 128:(t + 1) * 128],
                                ident_s[:])
                            yield
                            nc.vector.tensor_copy(
                                v_s[b][:, t * VBLK:t * VBLK + DH],
                                pst[:, 0:DH])
                            nc.vector.tensor_copy(
                                v_s[b][:, t * VBLK + DH + 1:
                                       t * VBLK + 2 * DH + 1],
                                pst[:, DH:2 * DH])
                            v_emitted[b] = t + 1

            # ---- output projection for one q-chunk of a batch ----
            def outproj(b, qc):
                # the very last chunk drains via the (by then idle) scalar
                # engine so the tail isn't serialized behind the DVE chain
                tail_chunk = (b == B - 1 and qc == NQC - 1)
                for t in range(qc * NTB // NQC, (qc + 1) * NTB // NQC):
                    o_sb = outpool.tile([128, 1024], F16, tag="outsb",
                                        name=f"o_sb{b}_{t}")
                    for ch in range(2):
                        acc = psppool.tile([128, 512], F32, tag="psP",
                                           name=f"psO{b}_{t}_{ch}")
                        nc.tensor.matmul(
                            acc[:],
                            ctxT_s[b][:, t * 128:(t + 1) * 128],
                            wo_s[:, ch * 512:(ch + 1) * 512],
                            start=True, stop=True)
                        yield
                        if tail_chunk:
                            nc.scalar.copy(
                                o_sb[:, ch * 512:(ch + 1) * 512], acc[:])
                        else:
                            nc.vector.tensor_copy(
                                o_sb[:, ch * 512:(ch + 1) * 512], acc[:])
                    nc.sync.dma_start(
                        out_d[b * S + t * 128:b * S + (t + 1) * 128, :],
                        o_sb[:])

            # ---- background PE work, gated by global kb position ----
            bg = []      # list of [gate_pos, generator]
            pos = [0]    # global kb counter: (b*NQC + qc)*NKB + kb

            def drive(n):
                pumps = 0
                while pumps < n:
                    for ent in bg:
                        if ent[0] <= pos[0]:
                            try:
                                next(ent[1])
                                pumps += 1
                            except StopIteration:
                                bg.remove(ent)
                            break
                    else:
                        return

            def drain_gated(max_gate):
                # force-emit (foreground) all bg entries at or below a gate
                for ent in [e for e in bg if e[0] <= max_gate]:
                    for _ in ent[1]:
                        pass
                    bg.remove(ent)

            def drain():
                while bg:
                    for _ in bg.pop(0)[1]:
                        pass

            # ---- deferred U emission ----
            pend = collections.deque()
            # number of v blocks whose SBUF copies have been EMITTED per
            # batch; a pending U may only be emitted once its v block's
            # write is in the instruction stream (else the read gets no
            # dependency and races at runtime).
            v_emitted = [0, 0]

            def can_pop():
                if not pend:
                    return False
                ent = pend[0]
                return v_emitted[ent[2]] > ent[4]

            def emit_entry(ent):
                psUt, e_sb, b, qc, kb = ent
                for h in range(HPC):
                    nc.tensor.matmul(
                        psUt[h][:],
                        v_s[b][:, kb * VBLK + h * (DH + 1):
                               kb * VBLK + (h + 1) * (DH + 1)],
                        e_sb[:, h * 512:(h + 1) * 512],
                        start=(kb == 0), stop=(kb == NKB - 1))
                if kb == NKB - 1:
                    chain(b, qc, psUt)

            def chain(b, qc, psUt):
                # softmax normalizer + ctx write for one finished q-chunk
                qsl = slice(qc * 512, (qc + 1) * 512)
                for h in range(HPC):
                    s_sb = recpool.tile([1, 512], F32, tag="r",
                                        name=f"s_sb{b}_{qc}_{h}")
                    nc.vector.tensor_copy(s_sb[:], psUt[h][DH:DH + 1, :])
                    rr = recpool.tile([1, 512], F32, tag="r",
                                      name=f"rr{b}_{qc}_{h}")
                    nc.vector.reciprocal_approx_fast(rr[:], s_sb[:])
                    Rb = rsbpool.tile([DH, 512], F32, tag="Rsb",
                                      name=f"Rb{b}_{qc}_{h}")
                    nc.gpsimd.partition_broadcast(Rb[:], rr[:])
                    nc.vector.tensor_mul(
                        ctxT_s[b][h * DH:(h + 1) * DH, qsl],
                        psUt[h][0:DH, :], Rb[:])
                # output projection becomes available for this q-chunk.
                # b0's outprojs run during attention(1) (gate at b1 start);
                # b1's are urgent (tail!) and jump the background queue.
                if b == 0:
                    bg.append([NQC * NKB, outproj(b, qc)])
                else:
                    # jump the queue, gated a few kb out so the first matmul
                    # never blocks the PE on the still-running chain
                    gate = pos[0] + 8 if qc < NQC - 1 else 0
                    bg.insert(0, [gate, outproj(b, qc)])

            # U-emission pacing: at "held" positions no U matmuls are
            # emitted (their v blocks aren't projected yet — popping early
            # would stall the in-order PE queue); elsewhere the backlog
            # drains at 2 pops/kb. Background work gets 3 slots/kb on held
            # positions (v projection needs the bandwidth), 1 otherwise.
            held = (set(range(0, 14)) | {16, 17} | set(range(20, 28))
                    | set(range(64, 72)))

            # ---- attention for one batch ----
            def attention(b):
                for qc in range(NQC):
                    qsl = slice(qc * 512, (qc + 1) * 512)
                    if qc > 0:
                        # foreground-project the next q chunk (data is there)
                        for _ in qproj(b, qc):
                            pass
                    psUt = [psupool.tile([DH + 1, 512], F32, tag="psU",
                                         name=f"psU{b}_{qc}_{h}")
                            for h in range(HPC)]
                    for kb in range(NKB):
                        psE = psapool.tile([128, 1024], F32, tag="psA",
                                           name=f"psE{b}_{qc}_{kb}")
                        for h in range(HPC):
                            rows = slice(64 * h, 64 * h + 64)
                            nc.tensor.matmul(
                                psE[:, h * 512:(h + 1) * 512],
                                kT_s[b][rows, kb * 128:(kb + 1) * 128],
                                qT_s[b][rows, qsl],
                                start=True, stop=True)
                        e_sb = epool.tile([128, 1024], F16, tag="E",
                                          name=f"e{b}_{qc}_{kb}")
                        nc.scalar.activation(
                            e_sb[:], psE[:],
                            mybir.ActivationFunctionType.Exp,
                            bias=maskb_s[:, b * NKB + kb:b * NKB + kb + 1],
                            scale=SCALE)
                        pend.append((psUt, e_sb, b, qc, kb))
                        g = pos[0]
                        pops = 0
                        if g not in held:
                            # at qc end, flush the steady-state backlog so
                            # the chain (whose DVE ops free the psU slots)
                            # is emitted a kb earlier
                            floor = 0 if kb == NKB - 1 else 3
                            cap = 3 if kb == NKB - 1 else 2
                            while (len(pend) > floor and pops < cap
                                   and can_pop()):
                                emit_entry(pend.popleft())
                                pops += 1
                        if pops == 0 or 56 <= g < 64:
                            # last stretch of b0: clear the b1-projection
                            # queue so the batch boundary has no burst
                            drive(3)
                        else:
                            drive(1)
                        # safety valve: the e-tile ring must not wrap onto
                        # entries whose U reads aren't emitted yet — push
                        # background (v-projection) emission until it can.
                        while len(pend) >= 22:
                            if can_pop():
                                emit_entry(pend.popleft())
                                continue
                            if not bg:
                                raise RuntimeError("U schedule wedged")
                            ent = bg[0]
                            try:
                                next(ent[1])
                            except StopIteration:
                                bg.remove(ent)
                        pos[0] = g + 1

            # ---- phase schedule ----
            # foreground: k-pp0 and the first q chunk of batch 0, so the
            # first score matmul fires as early as DMA allows (scores kb0-7
            # only need the first 1024 tokens of kT).
            for _ in kvproj(0, 0, "k"):
                pass
            for _ in qproj(0, 0):
                pass
            # background, in DMA-priority order (gates = global kb index)
            v1pp1_ent = [56, kvproj(1, 1, "v")]
            bg.append([0, kvproj(0, 1, "k")])
            bg.append([0, wv_ident_gen()])
            bg.append([9, kvproj(0, 0, "v")])
            bg.append([19, kvproj(0, 1, "v")])
            bg.append([24, kvproj(1, 0, "k")])
            bg.append([40, kvproj(1, 1, "k")])
            bg.append([48, qproj(1, 0)])
            bg.append([40, wo_gen()])
            bg.append([48, kvproj(1, 0, "v")])
            bg.append(v1pp1_ent)

            attention(0)
            # everything batch-1's first scores/U touch must be emitted
            # before attention(1) emits reads of it; v1-pp1 keeps streaming
            # in the background (its U consumers are held back by the
            # held positions and the v_emitted guard).
            drain_gated(48)
            v1pp1_ent[0] = 64
            attention(1)
            # finish all background emission FIRST (so every remaining v
            # write is in the stream), then flush pending U, then drain the
            # outprojs appended by the final chains.
            drain()
            while pend:
                emit_entry(pend.popleft())
            drain()
            if dbg:
                for i, src in enumerate((qT_s[1], kT_s[1], ctxT_s[0],
                                         ctxT_s[1])):
                    nc.sync.dma_start(dbg_d[:, i * S:(i + 1) * S], src[:])

    nc.compile()
    return nc


_NC_CACHE = {}
LAST_RESULT = {}


def kernel(**inputs):
    query = np.asarray(inputs["query"], np.float32)
    key = np.asarray(inputs["key"], np.float32)
    value = np.asarray(inputs["value"], np.float32)
    mask = np.asarray(inputs["mask"], np.float32)
    Wq = np.asarray(inputs["Wq"], np.float32)
    Wk = np.asarray(inputs["Wk"], np.float32)
    Wv = np.asarray(inputs["Wv"], np.float32)
    Wo = np.asarray(inputs["Wo"], np.float32)
    bq = np.asarray(inputs["bq"], np.float32)
    bk = np.asarray(inputs["bk"], np.float32)
    bv = np.asarray(inputs["bv"], np.float32)
    bo = np.asarray(inputs["bo"], np.float32)

    zero_bias = (not bq.any()) and (not bk.any()) and (not bv.any())

    qT = np.ascontiguousarray(query.reshape(T, D).T.astype(np.float16))
    kT = np.ascontiguousarray(key.reshape(T, D).T.astype(np.float16))
    vT = np.ascontiguousarray(value.reshape(T, D).T.astype(np.float16))
    # maskb[p, b*16+kb] = -1e9 * mask[b, 0, 0, kb*128+p]
    maskb = np.ascontiguousarray(
        (mask[:, 0, 0, :] * np.float32(-1e9))
        .reshape(B, S // 128, 128).transpose(2, 0, 1).reshape(128, -1))
    ident = np.eye(128, dtype=np.float16)

    def permw(W, cols):
        # [D, CW] -> [128, NKT*CW]: w[p, kt*CW+m] = W[kt*128+p, m]
        return np.ascontiguousarray(
            W[:, cols].astype(np.float16)
            .reshape(NKT, 128, CW).transpose(1, 0, 2).reshape(128, NKT * CW))

    in_maps = []
    for c in range(NCORES):
        cols = slice(CW * c, CW * (c + 1))
        in_maps.append({
            "qT": qT, "kT": kT, "vT": vT,
            "wq": permw(Wq, cols),
            "wk": permw(Wk, cols),
            "wv": permw(Wv, cols),
            "wo": np.ascontiguousarray(Wo[cols, :].astype(np.float16)),
            "bqkv": np.ascontiguousarray(
                np.stack([bq[cols], bk[cols], bv[cols]], axis=1)),
            "maskb": maskb,
            "ident": ident,
        })

    import os
    dbg = bool(os.environ.get("KERNEL_DEBUG_DUMP"))
    ck = (zero_bias, dbg)
    if ck not in _NC_CACHE:
        _NC_CACHE[ck] = build_nc(zero_bias=zero_bias, dbg=dbg)
    nc = _NC_CACHE[ck]

    trace = bool(os.environ.get("KERNEL_TRACE"))
    res = run_bass_kernel_spmd(nc, in_maps, core_ids=list(range(NCORES)),
                               trace=trace)
    LAST_RESULT["res"] = res
    out = np.zeros((T, D), np.float64)
    for c in range(NCORES):
        out += res.results[c]["out"].astype(np.float64)
    out = (out + bo.astype(np.float64)).astype(np.float32)
    return out.reshape(B, S, D)
